# revision 12
# baseline (speedup 1.0000x reference)
"""Bahdanau-attention kernel for Trainium2 (8 NeuronCores, data-parallel over batch).

reference math:
  energy = relu(concat([hidden bcast T, enc], -1) @ W.T + b)   # [B,T,D]
  scores = energy @ v                                          # [B,T]
  out    = softmax(scores, axis=T)[:, None, :]                 # [B,1,T]

Per-core kernel (4 batch elems, 8192 bt rows), fp8 DoubleRow GEMM:
  relu decomposition: v.relu(z) = (v/2).z + (v/2).|z| with z = W2 enc + hb,
  hb = hid @ W1.T + b. The (v/2).z part collapses to qe = enc @ (W2.T v/2)
  (host-exact) plus a per-batch constant that cancels in softmax; only |z|
  carries fp8 error (measured 1.13e-2 on HW, tolerance 2e-2).
  Host pre-transposes enc to [k, bt] fp8-e4m3 and W2.T (scaled by S_W) to
  fp8; no device transposes or cast-DMAs.
  GEMM: zT[d, bt] via DoubleRow fp8 matmuls (K=256 per MM, planes = k-tile
  pairs) at the DR roofline ~240ns/MM. bt-tiles are processed in groups of
  widths [1,1,2,4,4,2,1,1]: narrow lead-in groups start the PE as soon as
  the first 512KB of enc lands (DMA delivers ~512KB/2.6us after the ~7us
  engine preamble), keeping HAM warm; narrow tail groups let the last
  v-dot flushes interleave instead of trailing the GEMM.
  ACT applies |z/S_W + hb| -> bf16 into per-group energy slabs. v-dots:
  8 M=1 bf16 matmuls per bt-tile accumulated into a single PSUM row (fp8
  energy measured noticeably less accurate; col-group packing measured no
  faster). One DVE add folds in qe, ACT exp + segment accum, fp32 softmax.
"""
import numpy as np
import ml_dtypes
import concourse.mybir as mybir
import concourse.tile as tile
import concourse.bacc as bacc
from concourse import bass_utils

P = 128
B, T, D = 32, 2048, 1024
N_CORES = 8
NB = B // N_CORES            # 4 local batch elems
BT = NB * T                  # 8192 local rows
BTT = 512                    # bt-tile (columns of z^T)
N_BT = BT // BTT             # 16 bt-tiles
GWID = [1, 1, 2, 2, 2, 2, 2, 2, 1, 1]  # bt-tiles per group (never spans a batch elem)
DT = D // P                  # 8 d-tiles (output dim of W2)
KT = D // P                  # 8 k-tiles (contraction over enc features)
KP = KT // 2                 # 4 k-pair super-tiles (DoubleRow planes)
S_W = 32.0                   # fp8 W2 scale (undone in the ACT scale)
N_WARM = 6                   # dummy matmuls to warm the PE HAM while DMAs land

BF16, F32 = mybir.dt.bfloat16, mybir.dt.float32
F8 = mybir.dt.float8e4
DR = mybir.MatmulPerfMode.DoubleRow
ABS = mybir.ActivationFunctionType.Abs
EXP = mybir.ActivationFunctionType.Exp


def _build():
    nc = bacc.Bacc("TRN2", target_bir_lowering=False, debug=False)
    ENC = nc.dram_tensor("enc", [D, BT], F8, kind="ExternalInput").ap()
    W2T = nc.dram_tensor("w2t", [P, KT * D], F8, kind="ExternalInput").ap()
    HB = nc.dram_tensor("hb", [P, DT * NB], F32, kind="ExternalInput").ap()
    VT = nc.dram_tensor("vt", [P, DT], BF16, kind="ExternalInput").ap()
    QE = nc.dram_tensor("qe", [NB, T], F32, kind="ExternalInput").ap()
    OUT = nc.dram_tensor("out", [NB, T], F32, kind="ExternalOutput").ap()

    with tile.TileContext(nc) as tc, \
         tc.tile_pool(name="persist", bufs=1) as pp, \
         tc.tile_pool(name="encs", bufs=8) as eps, \
         tc.tile_pool(name="encb", bufs=8) as epb, \
         tc.tile_pool(name="e_s", bufs=3) as ebs, \
         tc.tile_pool(name="e_b", bufs=2) as ebb, \
         tc.tile_pool(name="ps_z", bufs=3, space="PSUM") as zp, \
         tc.tile_pool(name="ps_s", bufs=2, space="PSUM") as sp, \
         tc.tile_pool(name="sm", bufs=1) as smp:

        # ---- persistent small tensors ----
        w2 = [pp.tile([P, 2 * D], F8, name=f"w2_{k}") for k in range(KP)]
        hb = pp.tile([P, DT * NB], F32)   # col di*NB+b = (hid@W1.T)[b,d]+bias[d]
        vt = pp.tile([P, DT], BF16)       # col di = v[di*128:(di+1)*128]/2
        # batch elem bi lives on partition 32*bi (compute outputs need
        # 32-aligned partition bases)
        qe = pp.tile([P, T], F32)         # row 32*bi = host-exact linear part
        scores = pp.tile([P, T], F32)
        exs = pp.tile([P, T], F32)        # exp(scores), filled per segment
        part = pp.tile([P, T // BTT], F32)  # per-segment exp sums
        warm = pp.tile([P, BTT], BF16)

        g0 = [sum(GWID[:i]) for i in range(len(GWID))]  # first bt-tile of group

        # ---- enc tiles: per (group, k-pair): [128, 2 planes, W*BTT] ----
        enc_t = {}

        def load_group(gi, engs):
            w = GWID[gi]
            pool = eps if w <= 2 else epb
            for kp in range(KP):
                t_ = pool.tile([P, 2 * w * BTT], F8,
                               tag=f"enc{w}", bufs=8, name=f"enc{gi}_{kp}")
                for j in range(2):
                    engs[(2 * kp + j) % len(engs)].dma_start(
                        out=t_[:, j * w * BTT:(j + 1) * w * BTT],
                        in_=ENC[(2 * kp + j) * P:(2 * kp + j + 1) * P,
                                g0[gi] * BTT:(g0[gi] + w) * BTT])
                enc_t[(gi, kp)] = t_

        # DMA priority: lead-in enc groups on sync/gpsimd, w2 on scalar --
        # the only tensors gating the first matmuls; everything else after
        for gi in range(3):
            load_group(gi, [nc.sync, nc.gpsimd])
        for k in range(KP):
            nc.scalar.dma_start(out=w2[k], in_=W2T[:, k * 2 * D:(k + 1) * 2 * D])
        nc.scalar.dma_start(out=vt, in_=VT)
        nc.scalar.dma_start(out=hb, in_=HB)
        # PE warmup: HAM un-throttles after ~3.4us of sustained activity
        nc.vector.memset(warm, 0.0)
        for i in range(N_WARM):
            wps = sp.tile([P, BTT], F32, tag="s", name=f"warm{i}")
            nc.tensor.matmul(wps, warm[:, 0:P], warm, start=True, stop=True)
        for bi in range(NB):
            nc.scalar.dma_start(out=qe[32 * bi:32 * bi + 1, :],
                                in_=QE[bi:bi + 1, :])
        for gi in range(3, len(GWID)):
            load_group(gi, [nc.sync, nc.gpsimd, nc.scalar])

        # ---- softmax over T for one batch elem (scores row 32*bi) ----
        def softmax_row(bi):
            ssum = smp.tile([1, 1], F32, tag="ssum", name=f"ssum{bi}", bufs=NB)
            nc.vector.reduce_sum(ssum, part[32 * bi:32 * bi + 1, :],
                                 axis=mybir.AxisListType.X)
            rinv = smp.tile([1, 1], F32, tag="rinv", name=f"rinv{bi}", bufs=NB)
            nc.vector.reciprocal(rinv, ssum)
            o_sb = smp.tile([1, T], F32, tag="osb", name=f"osb{bi}", bufs=2)
            nc.vector.tensor_scalar_mul(o_sb, exs[32 * bi:32 * bi + 1, :],
                                        rinv[:, 0:1])
            nc.sync.dma_start(out=OUT[bi:bi + 1, :], in_=o_sb)

        # ---- dots for one bt-tile: 8 v-dots on the |z| slab, accumulated
        # into one PSUM row; DVE folds in qe; ACT exp + segment accum ----
        emap = {}                          # n -> list of (slab, col0) by di

        def flush_dots(n):
            bi, m = n // NB, n % NB
            toff = m * BTT
            ps = sp.tile([P, BTT], F32, tag="s", name=f"ps_s{n}")
            for i in range(DT):
                eg, c0 = emap[n][i]
                nc.tensor.matmul(
                    ps[0:1, :], vt[:, i:i + 1], eg[:, c0:c0 + BTT],
                    start=(i == 0), stop=(i == DT - 1))
            nc.vector.tensor_add(
                scores[32 * bi:32 * bi + 1, toff:toff + BTT],
                ps[0:1, :], qe[32 * bi:32 * bi + 1, toff:toff + BTT])
            nc.scalar.activation(
                exs[32 * bi:32 * bi + 1, toff:toff + BTT],
                scores[32 * bi:32 * bi + 1, toff:toff + BTT], EXP,
                bias=0.0, scale=1.0,
                accum_out=part[32 * bi:32 * bi + 1, m:m + 1])
            if m == NB - 1:
                softmax_row(bi)

        # ---- main GEMM: each group shares stationary weights across its
        # bt-tiles; z psums hold up to 2 bt-tiles so one ACT drains two;
        # pending dots interleave after odd di ----
        pending = []
        for gi, w in enumerate(GWID):
            bi = g0[gi] // NB
            pool = ebs if w <= 2 else ebb
            eg = pool.tile([P, DT * w * BTT], BF16,
                           tag=f"eb{w}", bufs=3, name=f"eg{gi}")
            for mi in range(w):
                emap[g0[gi] + mi] = [(eg, (di * w + mi) * BTT)
                                     for di in range(DT)]
            for di in range(DT):
                nzt = (w + 1) // 2
                zt = [zp.tile([P, 2 * BTT], F32, tag="z",
                              name=f"z{gi}_{di}_{pr}") for pr in range(nzt)]
                for kp in range(KP):
                    lhsT = w2[kp].rearrange(
                        "p (j d) -> p j d", j=2)[:, :, di * P:(di + 1) * P]
                    for mi in range(w):
                        rhs = enc_t[(gi, kp)].rearrange(
                            "p (j c) -> p j c",
                            j=2)[:, :, mi * BTT:(mi + 1) * BTT]
                        nc.tensor.matmul(
                            zt[mi // 2][:, (mi % 2) * BTT:(mi % 2 + 1) * BTT],
                            lhsT, rhs, perf_mode=DR,
                            start=(kp == 0), stop=(kp == KP - 1))
                for pr in range(nzt):
                    wpr = min(2, w - 2 * pr)
                    c0 = (di * w + 2 * pr) * BTT
                    nc.scalar.activation(
                        eg[:, c0:c0 + wpr * BTT], zt[pr][:, :wpr * BTT], ABS,
                        bias=hb[:, di * NB + bi:di * NB + bi + 1],
                        scale=1.0 / S_W)
                if di % 2 == 1 and pending:
                    flush_dots(pending.pop(0))
            pending.extend(range(g0[gi], g0[gi] + w))
        for n in pending:
            flush_dots(n)

    nc.compile()
    return nc


def make_in_maps(hidden, enc, W, b, v):
    """Per-core input dicts: batch-sharded enc (pre-transposed to [k, bt],
    fp8), replicated small tensors. hb = hid @ W1.T + b and the abs-trick
    linear part qe = enc @ (W2.T v/2) are computed exactly on host."""
    f8 = ml_dtypes.float8_e4m3
    W1, W2 = W[:, :D], W[:, D:]
    hb_all = (hidden.astype(np.float64) @ W1.astype(np.float64).T
              + b.astype(np.float64)).astype(np.float32)        # [B, D]
    vh = v.astype(np.float64) / 2
    q = vh @ W2.astype(np.float64)                              # [D]
    encf = np.asarray(enc, np.float32)
    qe_all = (encf.reshape(B * T, D).astype(np.float64) @ q) \
        .astype(np.float32).reshape(B, T)
    vt = np.ascontiguousarray(
        vh.astype(ml_dtypes.bfloat16).reshape(DT, P).T)         # [128, 8]
    w2s = (W2.T.astype(np.float64) * S_W).astype(np.float32)
    # w2t [128, KT*D]: col block kj holds W2.T[kj*128+p, :] (k-tile pairs are
    # adjacent blocks -> DoubleRow planes)
    w2t = np.ascontiguousarray(
        w2s.astype(f8).reshape(KT, P, D).transpose(1, 0, 2).reshape(P, KT * D))
    enc8 = encf.astype(f8)                                      # [B, T, D]
    in_maps = []
    for c in range(N_CORES):
        enc_c = np.ascontiguousarray(
            enc8[c * NB:(c + 1) * NB].reshape(BT, D).T)         # [D, BT]
        hb_c = np.ascontiguousarray(
            hb_all[c * NB:(c + 1) * NB].reshape(NB, DT, P)
            .transpose(2, 1, 0).reshape(P, DT * NB))            # [128, 32]
        qe_c = np.ascontiguousarray(qe_all[c * NB:(c + 1) * NB])  # [4, 2048]
        in_maps.append(dict(enc=enc_c, w2t=w2t, hb=hb_c, vt=vt, qe=qe_c))
    return in_maps


_NC_CACHE = []


def kernel(hidden, encoder_outputs, W, b, v):
    hidden = np.asarray(hidden, dtype=np.float32)
    enc = np.asarray(encoder_outputs, dtype=np.float32)
    W = np.asarray(W, dtype=np.float32)
    b = np.asarray(b, dtype=np.float32)
    v = np.asarray(v, dtype=np.float32)

    if not _NC_CACHE:
        _NC_CACHE.append(_build())
    nc = _NC_CACHE[0]

    in_maps = make_in_maps(hidden, enc, W, b, v)
    res = bass_utils.run_bass_kernel_spmd(nc, in_maps, core_ids=list(range(N_CORES)))
    scores = np.concatenate([res.results[c]["out"] for c in range(N_CORES)], axis=0)
    return scores[:, None, :].astype(np.float32)


# revision 20
# speedup vs baseline: 1.1788x; 1.1788x over previous
"""Bahdanau-attention kernel for Trainium2 (8 NeuronCores, data-parallel over batch).

reference math:
  energy = relu(concat([hidden bcast T, enc], -1) @ W.T + b)   # [B,T,D]
  scores = energy @ v                                          # [B,T]
  out    = softmax(scores, axis=T)[:, None, :]                 # [B,1,T]

Per-core kernel (4 batch elems, 8192 bt rows), fp8 DoubleRow GEMM:
  relu decomposition: v.relu(z) = (v/2).z + (v/2).|z| with z = W2 enc + hb,
  hb = hid @ W1.T + b. The (v/2).z part collapses to qe = enc @ (W2.T v/2)
  (host-exact) plus a per-batch constant that cancels in softmax; only |z|
  carries fp8 error (measured 1.13e-2 on HW, tolerance 2e-2).
  Host pre-transposes enc to [k, bt] fp8-e4m3 and W2.T (scaled by S_W) to
  fp8; no device transposes or cast-DMAs.
  GEMM: zT[d, bt] via DoubleRow fp8 matmuls (K=256 per MM, planes = k-tile
  pairs) at the DR roofline ~240ns/MM. bt-tiles are processed in groups of
  widths [1,1,2,4,4,2,1,1]: narrow lead-in groups start the PE as soon as
  the first 512KB of enc lands (DMA delivers ~512KB/2.6us after the ~7us
  engine preamble), keeping HAM warm; narrow tail groups let the last
  v-dot flushes interleave instead of trailing the GEMM.
  ACT applies |z/S_W + hb| -> bf16 into per-group energy slabs. v-dots:
  8 M=1 bf16 matmuls per bt-tile accumulated into a single PSUM row (fp8
  energy measured noticeably less accurate; col-group packing measured no
  faster). One DVE add folds in qe, ACT exp + segment accum, fp32 softmax.
"""
import numpy as np
import ml_dtypes
import concourse.mybir as mybir
import concourse.tile as tile
import concourse.bacc as bacc
from concourse import bass_utils

P = 128
B, T, D = 32, 2048, 1024
N_CORES = 8
NB = B // N_CORES            # 4 local batch elems
BT = NB * T                  # 8192 local rows
BTT = 512                    # bt-tile (columns of z^T)
N_BT = BT // BTT             # 16 bt-tiles
GWID = [1, 1, 2, 2, 2, 2, 2, 2, 1, 1]  # bt-tiles per group (never spans a batch elem)
DT = D // P                  # 8 d-tiles (output dim of W2)
KT = D // P                  # 8 k-tiles (contraction over enc features)
KP = KT // 2                 # 4 k-pair super-tiles (DoubleRow planes)
S_W = 32.0                   # fp8 W2 scale (undone in the ACT scale)
N_WARM = 10                  # dummy matmuls to warm the PE HAM while DMAs land

BF16, F32 = mybir.dt.bfloat16, mybir.dt.float32
F8 = mybir.dt.float8e4
DR = mybir.MatmulPerfMode.DoubleRow
ABS = mybir.ActivationFunctionType.Abs
EXP = mybir.ActivationFunctionType.Exp


def _build():
    nc = bacc.Bacc("TRN2", target_bir_lowering=False, debug=False)
    ENC = nc.dram_tensor("enc", [D, BT], F8, kind="ExternalInput").ap()
    W2T = nc.dram_tensor("w2t", [P, KT * D], F8, kind="ExternalInput").ap()
    HB = nc.dram_tensor("hb", [P, DT * NB], F32, kind="ExternalInput").ap()
    VT = nc.dram_tensor("vt", [P, DT], BF16, kind="ExternalInput").ap()
    QE = nc.dram_tensor("qe", [NB, T], F32, kind="ExternalInput").ap()
    OUT = nc.dram_tensor("out", [NB, T], F32, kind="ExternalOutput").ap()

    with tile.TileContext(nc) as tc, \
         tc.tile_pool(name="persist", bufs=1) as pp, \
         tc.tile_pool(name="encs", bufs=8) as eps, \
         tc.tile_pool(name="encb", bufs=8) as epb, \
         tc.tile_pool(name="e_s", bufs=3) as ebs, \
         tc.tile_pool(name="e_b", bufs=2) as ebb, \
         tc.tile_pool(name="ps_z", bufs=3, space="PSUM") as zp, \
         tc.tile_pool(name="ps_s", bufs=2, space="PSUM") as sp, \
         tc.tile_pool(name="sm", bufs=1) as smp:

        # ---- persistent small tensors ----
        w2 = [pp.tile([P, 2 * D], F8, name=f"w2_{k}") for k in range(KP)]
        hb = pp.tile([P, DT * NB], F32)   # col di*NB+b = (hid@W1.T)[b,d]+bias[d]
        vt = pp.tile([P, DT], BF16)       # col di = v[di*128:(di+1)*128]/2
        # batch elem bi lives on partition 32*bi (compute outputs need
        # 32-aligned partition bases)
        qe = pp.tile([P, T], F32)         # row 32*bi = host-exact linear part
        scores = pp.tile([P, T], F32)
        exs = pp.tile([P, T], F32)        # exp(scores), filled per segment
        part = pp.tile([P, T // BTT], F32)  # per-segment exp sums
        warm = pp.tile([P, BTT], BF16)

        g0 = [sum(GWID[:i]) for i in range(len(GWID))]  # first bt-tile of group

        # ---- enc tiles: per (group, k-pair): [128, 2 planes, W*BTT] ----
        enc_t = {}

        def load_group(gi, engs):
            # one trigger per k-pair covering both DoubleRow planes (trigger
            # issue costs ~650ns of engine time regardless of size)
            w = GWID[gi]
            pool = eps if w <= 2 else epb
            for kp in range(KP):
                t_ = pool.tile([P, 2 * w * BTT], F8,
                               tag=f"enc{w}", bufs=8, name=f"enc{gi}_{kp}")
                src = ENC[2 * kp * P:(2 * kp + 2) * P,
                          g0[gi] * BTT:(g0[gi] + w) * BTT]
                engs[kp % len(engs)].dma_start(
                    out=t_.rearrange("p (j c) -> p j c", j=2),
                    in_=src.rearrange("(j p) c -> p j c", p=P))
                enc_t[(gi, kp)] = t_

        # DMA priority: lead-in enc groups on sync/gpsimd, w2 on scalar --
        # the only tensors gating the first matmuls; everything else after
        for gi in range(2):
            load_group(gi, [nc.sync, nc.gpsimd])
        for k in range(KP):
            nc.scalar.dma_start(out=w2[k], in_=W2T[:, k * 2 * D:(k + 1) * 2 * D])
        nc.scalar.dma_start(out=vt, in_=VT)
        nc.scalar.dma_start(out=hb, in_=HB)
        # PE warmup: HAM un-throttles after ~3.4us of sustained activity
        nc.vector.memset(warm, 0.0)
        for i in range(N_WARM):
            wps = sp.tile([P, BTT], F32, tag="s", name=f"warm{i}")
            nc.tensor.matmul(wps, warm[:, 0:P], warm, start=True, stop=True)
        for gi in range(2, len(GWID)):
            load_group(gi, [nc.sync, nc.gpsimd, nc.scalar])
        for bi in range(NB):
            nc.scalar.dma_start(out=qe[32 * bi:32 * bi + 1, :],
                                in_=QE[bi:bi + 1, :])

        # ---- softmax over T for one batch elem (scores row 32*bi) ----
        def softmax_row(bi):
            ssum = smp.tile([1, 1], F32, tag="ssum", name=f"ssum{bi}", bufs=NB)
            nc.vector.reduce_sum(ssum, part[32 * bi:32 * bi + 1, :],
                                 axis=mybir.AxisListType.X)
            rinv = smp.tile([1, 1], F32, tag="rinv", name=f"rinv{bi}", bufs=NB)
            nc.vector.reciprocal(rinv, ssum)
            o_sb = smp.tile([1, T], F32, tag="osb", name=f"osb{bi}", bufs=2)
            nc.vector.tensor_scalar_mul(o_sb, exs[32 * bi:32 * bi + 1, :],
                                        rinv[:, 0:1])
            nc.sync.dma_start(out=OUT[bi:bi + 1, :], in_=o_sb)

        # ---- dots for one bt-tile: 8 v-dots on the |z| slab, accumulated
        # into one PSUM row; DVE folds in qe; ACT exp + segment accum ----
        emap = {}                          # n -> list of (slab, col0) by di

        def flush_dots(n):
            bi, m = n // NB, n % NB
            toff = m * BTT
            ps = sp.tile([P, BTT], F32, tag="s", name=f"ps_s{n}")
            for i in range(DT):
                eg, c0 = emap[n][i]
                nc.tensor.matmul(
                    ps[0:1, :], vt[:, i:i + 1], eg[:, c0:c0 + BTT],
                    start=(i == 0), stop=(i == DT - 1))
            nc.vector.tensor_add(
                scores[32 * bi:32 * bi + 1, toff:toff + BTT],
                ps[0:1, :], qe[32 * bi:32 * bi + 1, toff:toff + BTT])
            nc.scalar.activation(
                exs[32 * bi:32 * bi + 1, toff:toff + BTT],
                scores[32 * bi:32 * bi + 1, toff:toff + BTT], EXP,
                bias=0.0, scale=1.0,
                accum_out=part[32 * bi:32 * bi + 1, m:m + 1])
            if m == NB - 1:
                softmax_row(bi)

        # ---- main GEMM: each group shares stationary weights across its
        # bt-tiles; z psums hold up to 2 bt-tiles so one ACT drains two;
        # pending dots interleave after odd di ----
        pending = []
        for gi, w in enumerate(GWID):
            bi = g0[gi] // NB
            pool = ebs if w <= 2 else ebb
            eg = pool.tile([P, DT * w * BTT], BF16,
                           tag=f"eb{w}", bufs=3, name=f"eg{gi}")
            for mi in range(w):
                emap[g0[gi] + mi] = [(eg, (di * w + mi) * BTT)
                                     for di in range(DT)]
            for di in range(DT):
                nzt = (w + 1) // 2
                zt = [zp.tile([P, 2 * BTT], F32, tag="z",
                              name=f"z{gi}_{di}_{pr}") for pr in range(nzt)]
                for kp in range(KP):
                    lhsT = w2[kp].rearrange(
                        "p (j d) -> p j d", j=2)[:, :, di * P:(di + 1) * P]
                    for mi in range(w):
                        rhs = enc_t[(gi, kp)].rearrange(
                            "p (j c) -> p j c",
                            j=2)[:, :, mi * BTT:(mi + 1) * BTT]
                        nc.tensor.matmul(
                            zt[mi // 2][:, (mi % 2) * BTT:(mi % 2 + 1) * BTT],
                            lhsT, rhs, perf_mode=DR,
                            start=(kp == 0), stop=(kp == KP - 1))
                for pr in range(nzt):
                    wpr = min(2, w - 2 * pr)
                    c0 = (di * w + 2 * pr) * BTT
                    nc.scalar.activation(
                        eg[:, c0:c0 + wpr * BTT], zt[pr][:, :wpr * BTT], ABS,
                        bias=hb[:, di * NB + bi:di * NB + bi + 1],
                        scale=1.0 / S_W)
                if di % 2 == 1 and pending:
                    flush_dots(pending.pop(0))
            pending.extend(range(g0[gi], g0[gi] + w))
        for n in pending:
            flush_dots(n)

    nc.compile()
    return nc


def make_in_maps(hidden, enc, W, b, v):
    """Per-core input dicts: batch-sharded enc (pre-transposed to [k, bt],
    fp8), replicated small tensors. hb = hid @ W1.T + b and the abs-trick
    linear part qe = enc @ (W2.T v/2) are computed exactly on host."""
    f8 = ml_dtypes.float8_e4m3
    W1, W2 = W[:, :D], W[:, D:]
    hb_all = (hidden.astype(np.float64) @ W1.astype(np.float64).T
              + b.astype(np.float64)).astype(np.float32)        # [B, D]
    vh = v.astype(np.float64) / 2
    q = vh @ W2.astype(np.float64)                              # [D]
    encf = np.asarray(enc, np.float32)
    qe_all = (encf.reshape(B * T, D).astype(np.float64) @ q) \
        .astype(np.float32).reshape(B, T)
    vt = np.ascontiguousarray(
        vh.astype(ml_dtypes.bfloat16).reshape(DT, P).T)         # [128, 8]
    w2s = (W2.T.astype(np.float64) * S_W).astype(np.float32)
    # w2t [128, KT*D]: col block kj holds W2.T[kj*128+p, :] (k-tile pairs are
    # adjacent blocks -> DoubleRow planes)
    w2t = np.ascontiguousarray(
        w2s.astype(f8).reshape(KT, P, D).transpose(1, 0, 2).reshape(P, KT * D))
    enc8 = encf.astype(f8)                                      # [B, T, D]
    in_maps = []
    for c in range(N_CORES):
        enc_c = np.ascontiguousarray(
            enc8[c * NB:(c + 1) * NB].reshape(BT, D).T)         # [D, BT]
        hb_c = np.ascontiguousarray(
            hb_all[c * NB:(c + 1) * NB].reshape(NB, DT, P)
            .transpose(2, 1, 0).reshape(P, DT * NB))            # [128, 32]
        qe_c = np.ascontiguousarray(qe_all[c * NB:(c + 1) * NB])  # [4, 2048]
        in_maps.append(dict(enc=enc_c, w2t=w2t, hb=hb_c, vt=vt, qe=qe_c))
    return in_maps


_NC_CACHE = []


def kernel(hidden, encoder_outputs, W, b, v):
    hidden = np.asarray(hidden, dtype=np.float32)
    enc = np.asarray(encoder_outputs, dtype=np.float32)
    W = np.asarray(W, dtype=np.float32)
    b = np.asarray(b, dtype=np.float32)
    v = np.asarray(v, dtype=np.float32)

    if not _NC_CACHE:
        _NC_CACHE.append(_build())
    nc = _NC_CACHE[0]

    in_maps = make_in_maps(hidden, enc, W, b, v)
    res = bass_utils.run_bass_kernel_spmd(nc, in_maps, core_ids=list(range(N_CORES)))
    scores = np.concatenate([res.results[c]["out"] for c in range(N_CORES)], axis=0)
    return scores[:, None, :].astype(np.float32)
